# revision 16
# baseline (speedup 1.0000x reference)
"""Trainium2 Bass kernel for an R-GCN-style GCN layer (basis decomposition).

Reference computation (per relation r, with W_r = sum_b coeff[r,b] * basis[b]):
    out = sum_r segment_sum(inp[src_r] * val_r, dst_r) @ W_r + sum_r bias[r]

Algebraic restructure (4 basis accumulators instead of 16 relation matmuls):
    out[d] = sum_b G_b[d] @ basis[b] + bias_sum
    G_b[d] = sum_{edges e: dst_e = d} (coeff[r_e, b] * val_e) * inp[src_e]

Distribution: output nodes are sharded 8 ways (12500 rows/core); no
cross-core communication.

Host-side layout (pure data movement, no arithmetic on features): the edge
structure is static, so the host pre-arranges inp[src_e] (bf16) into each
core's per-chunk layout with one fancy-index. The device then STREAMS the
edge features with large contiguous HWDGE DMAs instead of 200k+ per-row
SWDGE gather descriptors (the Q7 descriptor generator caps per-row gathers
at ~8 ns/row = 1.7 ms/core, 5x above the byte roofline).

Per-core static structure (identical across cores, SPMD):
  - Host balancer packs the core's 12500 dst nodes into 416 groups of <=32
    nodes (104 blocks of 4 groups, 26 superblocks of 4 blocks) such that
    every group holds <=512 edges -> exactly 4 K=128 chunks per group,
    64 chunks per SB. Padding slots are zero rows.
  - Per SB: one contiguous 2 MB x-stream DMA [128, 64 chunks, 128 feat].
  - Masks are built in TWO big bf16 DVE ops per SB (both 2x-mode eligible:
    all operands 2-byte with stride-1 last dims, chunk-minor layout):
      eq[p, n, c]      = (iota_rep[p, n, c] == ldst[p, c])
      mask[p, b, n, c] = eq[p, _, n, c] * w4T[p, b, _, c]
  - Per chunk c: one bf16 matmul gT[f, (q, b, n)] += X_c^T @ mask[:, :, :, c]
    into the block's PSUM bank (fp32 accumulate).
  - Per block: 4 bf16 basis matmuls outT[fout, (q, n)] += basis_b^T @ gT_b,
    bias fused into the PSUM->SBUF copy on the scalar engine; out stores
    ride the Activation engine's HWDGE so the Sync engine only prefetches.

Output is produced transposed per block ([fout, node]) and the host maps
(block, slot) -> node id via the balancer's permutation.
"""
import os
import sys

for _p in ("/opt/trn_rl_repo", "/root/.axon_site/_ro/trn_rl_repo"):
    if os.path.isdir(_p) and _p not in sys.path:
        sys.path.insert(0, _p)

import numpy as np
import ml_dtypes

import concourse.bass as bass
import concourse.tile as tile
from concourse import bacc, mybir
from concourse.bass_utils import run_bass_kernel_spmd

BF16NP = ml_dtypes.bfloat16

# ---------------- problem constants (hardcoded from spec) ----------------
NN = 100000          # nodes
F = 128              # feature dim (in == out)
NB = 4               # bases
NREL = 16            # relations
NCORES = 8
NS = NN // NCORES    # dst nodes per core (12500)

GROUP = 16           # dst nodes per group
GPB = 8              # groups per block
BLOCK = GROUP * GPB  # 128 dst nodes per block
NBLK = 100           # blocks (800 groups of <=16 nodes; 12800 slots >= 12500)
BPS = 4              # blocks per superblock
NSB = NBLK // BPS    # 26 superblocks

CPG = 2              # chunks per group (cap 256 edges/group)
CAP = CPG * 128      # 512 edge slots per group
CPS = BPS * GPB * CPG  # 64 chunks per SB
META_COLS = CPS + NB * CPS  # 320 bf16 cols per SB: [ldst: 64][w4T: 256]

F32 = mybir.dt.float32
BF16 = mybir.dt.bfloat16

_compiled = {}


def _build_program():
    nc = bacc.Bacc(
        "TRN2",
        target_bir_lowering=False,
        debug=False,
        enable_asserts=False,
        num_devices=NCORES,
    )

    xexp = nc.dram_tensor("xexp", [NSB, 128, CPS * F], BF16, kind="ExternalInput")
    basisw = nc.dram_tensor("basisw", [NB, F, F], BF16, kind="ExternalInput")
    biasw = nc.dram_tensor("biasw", [F, 1], F32, kind="ExternalInput")
    # iota_rep[p, n, c] = n  (constant, chunk-minor so DVE ops stay 2x-mode)
    iota = nc.dram_tensor("iota", [128, GROUP * CPS], BF16, kind="ExternalInput")
    meta = nc.dram_tensor("meta", [128, NSB * META_COLS], BF16, kind="ExternalInput")
    outT = nc.dram_tensor("outT", [NBLK, F, BLOCK], F32, kind="ExternalOutput")

    with tile.TileContext(nc) as tc:
        with (
            tc.tile_pool(name="const", bufs=1) as const,
            tc.tile_pool(name="xg", bufs=5) as xg,
            tc.tile_pool(name="metap", bufs=4) as metap,
            tc.tile_pool(name="eqp", bufs=2) as eqp,
            tc.tile_pool(name="msk", bufs=4) as mskp,
            tc.tile_pool(name="gt", bufs=3) as gtp,
            tc.tile_pool(name="ot", bufs=3) as otp,
            tc.tile_pool(name="psg", bufs=6, space="PSUM") as psg,
            tc.tile_pool(name="pso", bufs=2, space="PSUM") as pso,
        ):
            # ---- constants
            # constants load off the Sync engine so the per-SB x/meta stream
            # starts issuing immediately (Sync is in-order).
            iota_t = const.tile([128, GROUP, CPS], BF16)
            nc.scalar.dma_start(
                out=iota_t[:], in_=iota[:, :].rearrange("p (n c) -> p n c", n=GROUP)
            )
            basis_t = const.tile([F, NB * F], BF16)
            for b in range(NB):
                nc.scalar.dma_start(
                    out=basis_t[:, b * F : (b + 1) * F], in_=basisw[b, :, :]
                )
            # bias column: host ships sum_r bias[r] directly
            bias_col = const.tile([F, 1], F32)
            nc.scalar.dma_start(out=bias_col[:], in_=biasw[:, :])

            def basis_phase(sb, gtall):
                # 4 N=512 matmuls spanning all 4 blocks (basis phase runs one
                # SB delayed, so the gt copies are long since done).
                ot_sb = otp.tile([F, BPS * BLOCK], F32, tag="ot", name=f"ot{sb}")
                ot_ps = pso.tile([F, BPS * BLOCK], F32, tag="otp", name=f"otp{sb}")
                gt_v = gtall[:].rearrange(
                    "p (j q b n) -> p j q b n", j=BPS, q=GPB, b=NB
                )
                for bb in range(NB):
                    nc.tensor.matmul(
                        ot_ps[:].rearrange("p (j q n) -> p j q n", j=BPS, q=GPB),
                        lhsT=basis_t[:, bb * F : (bb + 1) * F],
                        rhs=gt_v[:, :, :, bb, :],
                        start=(bb == 0),
                        stop=(bb == NB - 1),
                    )
                nc.scalar.activation(
                    ot_sb[:],
                    ot_ps[:],
                    mybir.ActivationFunctionType.Identity,
                    bias=bias_col[:],
                )
                nc.scalar.dma_start(
                    out=outT[sb * BPS : (sb + 1) * BPS, :, :].rearrange(
                        "j f n -> f j n"
                    ),
                    in_=ot_sb[:].rearrange("p (j n) -> p j n", j=BPS),
                )

            pending = None
            for sb in range(NSB):
                # ---- edge features: one contiguous 2 MB stream per SB
                x_t = xg.tile([128, CPS, F], BF16, tag="x")
                xv = xexp[sb, :, :].rearrange("p (c f) -> p c f", f=F)
                half = CPS // 2
                nc.sync.dma_start(out=x_t[:, :half, :], in_=xv[:, :half, :])
                nc.sync.dma_start(out=x_t[:, half:, :], in_=xv[:, half:, :])
                meta_t = metap.tile([128, META_COLS], BF16)
                nc.sync.dma_start(
                    out=meta_t[:], in_=meta[:, sb * META_COLS : (sb + 1) * META_COLS]
                )
                ldst_s = meta_t[:, 0:CPS]
                w4t_s = meta_t[:, CPS:META_COLS].rearrange("p (b c) -> p b c", b=NB)

                # ---- masks: two big 2x-mode DVE ops
                eq_t = eqp.tile([128, GROUP, CPS], BF16)
                nc.vector.tensor_tensor(
                    eq_t[:],
                    iota_t[:],
                    ldst_s[:, None, :].to_broadcast([128, GROUP, CPS]),
                    mybir.AluOpType.is_equal,
                )
                msk_t = mskp.tile([128, NB, GROUP, CPS], BF16, tag="m")
                nc.vector.tensor_tensor(
                    msk_t[:],
                    eq_t[:, None, :, :].to_broadcast([128, NB, GROUP, CPS]),
                    w4t_s[:, :, None, :].to_broadcast([128, NB, GROUP, CPS]),
                    mybir.AluOpType.mult,
                )

                gt_ps = [
                    psg.tile([F, GPB * NB * GROUP], F32, tag="g", name=f"gt{b}")
                    for b in range(BPS)
                ]

                # ---- chunk matmuls. chunk col layout: c = bucket*CPG + k,
                # bucket = b*GPB + q. start=True arms a pending-zero for the
                # whole 2KB bank on trn2: exactly once per block bank.
                for cis in range(BPS * GPB):
                    b, q = cis // GPB, cis % GPB
                    for k in range(CPG):
                        col = cis * CPG + k
                        nc.tensor.matmul(
                            gt_ps[b][:, q * (NB * GROUP) : (q + 1) * (NB * GROUP)],
                            lhsT=x_t[:, col, :],
                            rhs=msk_t[:, :, :, col],
                            start=(q == 0 and k == 0),
                            stop=(q == GPB - 1 and k == CPG - 1),
                            skip_group_check=True,
                        )

                # ---- PSUM drain: copies now, basis matmuls one SB later so
                # the PE rolls straight into the next SB's mask matmuls.
                gtall = gtp.tile(
                    [F, BPS * GPB * NB * GROUP], BF16, tag="gts", name=f"gtall{sb}"
                )
                GW = GPB * NB * GROUP
                for b in range(BPS):
                    nc.scalar.copy(gtall[:, b * GW : (b + 1) * GW], gt_ps[b][:])
                if pending is not None:
                    basis_phase(pending[0], pending[1])
                pending = (sb, gtall)
            basis_phase(pending[0], pending[1])

    nc.compile()
    return nc


def _balance(tot):
    """Pack NS nodes (total degrees tot [NS]) into NBLK*GPB groups of <=32
    nodes with per-group load <= CAP. Greedy LPT."""
    G = NBLK * GPB
    order = np.argsort(-tot, kind="stable")
    loads = np.zeros(G, np.int64)
    counts = np.zeros(G, np.int32)
    assign = np.empty(tot.shape[0], np.int32)
    slot = np.empty(tot.shape[0], np.int32)
    for n in order:
        masked = np.where(counts < GROUP, loads, 1 << 40)
        g = int(np.argmin(masked))
        assign[n] = g
        slot[n] = counts[g]
        loads[g] += tot[n]
        counts[g] += 1
    assert loads.max() <= CAP, f"group overflow: {loads.max()} > {CAP}"
    return assign, slot


def _preprocess(inp, basis_coeff, edge_val, edge_src, edge_dst):
    """Pack edges into the static (SB, chunk, slot) structure and pre-arrange
    the bf16 edge features. Returns per-core
    (xexp [NSB, 128, CPS*F] bf16, meta [128, NSB*META_COLS] bf16,
     pos2node [NBLK*BLOCK] int64)."""
    src = np.ascontiguousarray(edge_src).ravel().astype(np.int64)
    dst = np.ascontiguousarray(edge_dst).ravel().astype(np.int64)
    val = np.ascontiguousarray(edge_val).ravel().astype(np.float32)
    rel = np.repeat(np.arange(NREL, dtype=np.int64), edge_src.shape[1])
    coeff = np.asarray(basis_coeff, dtype=np.float32)  # [NREL, NB]
    inp_b = inp.astype(BF16NP)

    core = dst // NS
    per_core = []
    for c in range(NCORES):
        msel = core == c
        s_ = src[msel]
        dl = (dst[msel] - c * NS).astype(np.int64)
        v = val[msel]
        r = rel[msel]

        tot = np.bincount(dl, minlength=NS)
        assign, slot = _balance(tot)

        g = assign[dl]                           # group 0..415
        n = slot[dl].astype(np.float32)          # node slot in group, 0..31

        order = np.argsort(g, kind="stable")
        s_, v, r, n, g = (a[order] for a in (s_, v, r, n, g))
        ngr = NBLK * GPB
        cnt = np.bincount(g, minlength=ngr)
        starts = np.zeros(ngr + 1, dtype=np.int64)
        np.cumsum(cnt, out=starts[1:])
        pos = np.arange(len(s_)) - starts[g]     # 0..CAP-1 within group
        k = pos // 128                           # sub-chunk within group
        p = pos % 128                            # slot within chunk

        j = g // GPB                             # block
        q = g % GPB                              # group within block
        sbi = j // BPS                           # superblock
        col = ((j % BPS) * GPB + q) * CPG + k    # chunk col in SB, 0..63

        # ---- pre-arranged edge features (zero rows for padding slots)
        xexp_c = np.zeros((NSB, 128, CPS, F), dtype=BF16NP)
        xexp_c[sbi, p, col] = inp_b[s_]
        xexp_c = xexp_c.reshape(NSB, 128, CPS * F)

        # ---- meta [NSB, 128, META_COLS] bf16: [ldst: CPS][w4T: NB*CPS]
        mldst = np.zeros((NSB, 128, CPS), dtype=np.float32)
        mw4 = np.zeros((NSB, 128, NB, CPS), dtype=np.float32)
        mldst[sbi, p, col] = n
        mw4[sbi, p, :, col] = coeff[r] * v[:, None]
        meta_c = np.concatenate(
            [mldst, mw4.reshape(NSB, 128, NB * CPS)], axis=2
        ).astype(BF16NP)
        meta_c = np.ascontiguousarray(
            meta_c.transpose(1, 0, 2).reshape(128, NSB * META_COLS)
        )

        # ---- output permutation: (block j, q*32+n) -> node id
        pos2node = np.full(NBLK * BLOCK, -1, np.int64)
        nodes = np.arange(NS, dtype=np.int64)
        jn = assign[nodes] // GPB
        qn = assign[nodes] % GPB
        pos2node[jn * BLOCK + qn * GROUP + slot[nodes]] = nodes
        per_core.append((xexp_c, meta_c, pos2node))
    return per_core


def kernel(inp, basis_weights, basis_coeff, bias, edge_val, edge_src, edge_dst):
    inp = np.ascontiguousarray(np.asarray(inp, dtype=np.float32))
    basis_weights = np.ascontiguousarray(np.asarray(basis_weights, dtype=np.float32))
    basis_coeff = np.asarray(basis_coeff, dtype=np.float32)
    bias = np.ascontiguousarray(np.asarray(bias, dtype=np.float32))

    if "nc" not in _compiled:
        _compiled["nc"] = _build_program()
    nc = _compiled["nc"]

    per_core = _preprocess(inp, basis_coeff, edge_val, edge_src, edge_dst)
    iota_np = np.ascontiguousarray(
        np.broadcast_to(
            np.arange(GROUP, dtype=np.float32)[None, :, None], (128, GROUP, CPS)
        ).reshape(128, GROUP * CPS).astype(BF16NP)
    )
    basis_b = np.ascontiguousarray(basis_weights.astype(BF16NP))

    in_maps = []
    for c in range(NCORES):
        xexp_c, meta_c, _ = per_core[c]
        in_maps.append(
            {
                "xexp": xexp_c,
                "basisw": basis_b,
                "biasw": np.ascontiguousarray(bias.sum(0)[:, None]),
                "iota": iota_np,
                "meta": meta_c,
            }
        )

    res = run_bass_kernel_spmd(nc, in_maps, list(range(NCORES)))
    _compiled["last_results"] = res

    out = np.empty((NN, F), dtype=np.float32)
    for c in range(NCORES):
        oT = np.asarray(res.results[c]["outT"])  # [NBLK, F, BLOCK]
        rows = oT.transpose(0, 2, 1).reshape(NBLK * BLOCK, F)
        pos2node = per_core[c][2]
        valid = pos2node >= 0
        out[c * NS + pos2node[valid]] = rows[valid]
    return out


# revision 17
# speedup vs baseline: 1.0590x; 1.0590x over previous
"""Trainium2 Bass kernel for an R-GCN-style GCN layer (basis decomposition).

Reference computation (per relation r, with W_r = sum_b coeff[r,b] * basis[b]):
    out = sum_r segment_sum(inp[src_r] * val_r, dst_r) @ W_r + sum_r bias[r]

Algebraic restructure (4 basis accumulators instead of 16 relation matmuls):
    out[d] = sum_b G_b[d] @ basis[b] + bias_sum
    G_b[d] = sum_{edges e: dst_e = d} (coeff[r_e, b] * val_e) * inp[src_e]

Distribution: output nodes are sharded 8 ways (12500 rows/core); no
cross-core communication.

Host-side layout (pure data movement, no arithmetic on features): the edge
structure is static, so the host pre-arranges inp[src_e] (bf16) into each
core's per-chunk layout with one fancy-index. The device then STREAMS the
edge features with large contiguous HWDGE DMAs instead of 200k+ per-row
SWDGE gather descriptors (the Q7 descriptor generator caps per-row gathers
at ~8 ns/row = 1.7 ms/core, 5x above the byte roofline).

Per-core static structure (identical across cores, SPMD):
  - Host balancer packs the core's 12500 dst nodes into 416 groups of <=32
    nodes (104 blocks of 4 groups, 26 superblocks of 4 blocks) such that
    every group holds <=512 edges -> exactly 4 K=128 chunks per group,
    64 chunks per SB. Padding slots are zero rows.
  - Per SB: one contiguous 2 MB x-stream DMA [128, 64 chunks, 128 feat].
  - Masks are built in TWO big bf16 DVE ops per SB (both 2x-mode eligible:
    all operands 2-byte with stride-1 last dims, chunk-minor layout):
      eq[p, n, c]      = (iota_rep[p, n, c] == ldst[p, c])
      mask[p, b, n, c] = eq[p, _, n, c] * w4T[p, b, _, c]
  - Per chunk c: one bf16 matmul gT[f, (q, b, n)] += X_c^T @ mask[:, :, :, c]
    into the block's PSUM bank (fp32 accumulate).
  - Per block: 4 bf16 basis matmuls outT[fout, (q, n)] += basis_b^T @ gT_b,
    bias fused into the PSUM->SBUF copy on the scalar engine; out stores
    ride the Activation engine's HWDGE so the Sync engine only prefetches.

Output is produced transposed per block ([fout, node]) and the host maps
(block, slot) -> node id via the balancer's permutation.
"""
import os
import sys

for _p in ("/opt/trn_rl_repo", "/root/.axon_site/_ro/trn_rl_repo"):
    if os.path.isdir(_p) and _p not in sys.path:
        sys.path.insert(0, _p)

import numpy as np
import ml_dtypes

import concourse.bass as bass
import concourse.tile as tile
from concourse import bacc, mybir
from concourse.bass_utils import run_bass_kernel_spmd

BF16NP = ml_dtypes.bfloat16

# ---------------- problem constants (hardcoded from spec) ----------------
NN = 100000          # nodes
F = 128              # feature dim (in == out)
NB = 4               # bases
NREL = 16            # relations
NCORES = 8
NS = NN // NCORES    # dst nodes per core (12500)

GROUP = 16           # dst nodes per group
GPB = 8              # groups per block
BLOCK = GROUP * GPB  # 128 dst nodes per block
NBLK = 100           # blocks (800 groups of <=16 nodes; 12800 slots >= 12500)
BPS = 4              # blocks per superblock
NSB = NBLK // BPS    # 26 superblocks

CPG = 2              # chunks per group (cap 256 edges/group)
CAP = CPG * 128      # 512 edge slots per group
CPS = BPS * GPB * CPG  # 64 chunks per SB
META_COLS = CPS + NB * CPS  # 320 bf16 cols per SB: [ldst: 64][w4T: 256]

F32 = mybir.dt.float32
BF16 = mybir.dt.bfloat16

_compiled = {}


def _build_program():
    nc = bacc.Bacc(
        "TRN2",
        target_bir_lowering=False,
        debug=False,
        enable_asserts=False,
        num_devices=NCORES,
    )

    xexp = nc.dram_tensor("xexp", [NSB, 128, CPS * F], BF16, kind="ExternalInput")
    basisw = nc.dram_tensor("basisw", [NB, F, F], BF16, kind="ExternalInput")
    biasw = nc.dram_tensor("biasw", [F, 1], F32, kind="ExternalInput")
    # iota_rep[p, n, c] = n  (constant, chunk-minor so DVE ops stay 2x-mode)
    iota = nc.dram_tensor("iota", [128, GROUP * CPS], BF16, kind="ExternalInput")
    meta = nc.dram_tensor("meta", [128, NSB * META_COLS], BF16, kind="ExternalInput")
    outT = nc.dram_tensor("outT", [NBLK, F, BLOCK], F32, kind="ExternalOutput")

    with tile.TileContext(nc) as tc:
        with (
            tc.tile_pool(name="const", bufs=1) as const,
            tc.tile_pool(name="xg", bufs=4) as xg,
            tc.tile_pool(name="metap", bufs=4) as metap,
            tc.tile_pool(name="eqp", bufs=2) as eqp,
            tc.tile_pool(name="msk", bufs=4) as mskp,
            tc.tile_pool(name="gt", bufs=9) as gtp,
            tc.tile_pool(name="ot", bufs=3) as otp,
            tc.tile_pool(name="psg", bufs=6, space="PSUM") as psg,
            tc.tile_pool(name="pso", bufs=2, space="PSUM") as pso,
        ):
            # ---- constants
            iota_t = const.tile([128, GROUP, CPS], BF16)
            nc.sync.dma_start(
                out=iota_t[:], in_=iota[:, :].rearrange("p (n c) -> p n c", n=GROUP)
            )
            basis_t = const.tile([F, NB * F], BF16)
            for b in range(NB):
                nc.sync.dma_start(
                    out=basis_t[:, b * F : (b + 1) * F], in_=basisw[b, :, :]
                )
            # bias column: host ships sum_r bias[r] directly
            bias_col = const.tile([F, 1], F32)
            nc.sync.dma_start(out=bias_col[:], in_=biasw[:, :])

            def basis_phase(sb, gt_sbs):
                ot_sb = otp.tile([F, BPS * BLOCK], F32, tag="ot", name=f"ot{sb}")
                for b in range(BPS):
                    ot_ps = pso.tile([F, BLOCK], F32, tag="otp", name=f"otp{sb}_{b}")
                    gt_v = gt_sbs[b][:].rearrange(
                        "p (q b n) -> p q b n", q=GPB, b=NB
                    )
                    for bb in range(NB):
                        nc.tensor.matmul(
                            ot_ps[:].rearrange("p (q n) -> p q n", q=GPB),
                            lhsT=basis_t[:, bb * F : (bb + 1) * F],
                            rhs=gt_v[:, :, bb, :],
                            start=(bb == 0),
                            stop=(bb == NB - 1),
                        )
                    nc.scalar.activation(
                        ot_sb[:, b * BLOCK : (b + 1) * BLOCK],
                        ot_ps[:],
                        mybir.ActivationFunctionType.Identity,
                        bias=bias_col[:],
                    )
                nc.scalar.dma_start(
                    out=outT[sb * BPS : (sb + 1) * BPS, :, :].rearrange(
                        "j f n -> f j n"
                    ),
                    in_=ot_sb[:].rearrange("p (j n) -> p j n", j=BPS),
                )

            pending = None
            for sb in range(NSB):
                # ---- edge features: one contiguous 2 MB stream per SB
                x_t = xg.tile([128, CPS, F], BF16, tag="x")
                xv = xexp[sb, :, :].rearrange("p (c f) -> p c f", f=F)
                half = CPS // 2
                nc.sync.dma_start(out=x_t[:, :half, :], in_=xv[:, :half, :])
                nc.sync.dma_start(out=x_t[:, half:, :], in_=xv[:, half:, :])
                meta_t = metap.tile([128, META_COLS], BF16)
                nc.sync.dma_start(
                    out=meta_t[:], in_=meta[:, sb * META_COLS : (sb + 1) * META_COLS]
                )
                ldst_s = meta_t[:, 0:CPS]
                w4t_s = meta_t[:, CPS:META_COLS].rearrange("p (b c) -> p b c", b=NB)

                # ---- masks: two big 2x-mode DVE ops
                eq_t = eqp.tile([128, GROUP, CPS], BF16)
                nc.vector.tensor_tensor(
                    eq_t[:],
                    iota_t[:],
                    ldst_s[:, None, :].to_broadcast([128, GROUP, CPS]),
                    mybir.AluOpType.is_equal,
                )
                msk_t = mskp.tile([128, NB, GROUP, CPS], BF16, tag="m")
                nc.vector.tensor_tensor(
                    msk_t[:],
                    eq_t[:, None, :, :].to_broadcast([128, NB, GROUP, CPS]),
                    w4t_s[:, :, None, :].to_broadcast([128, NB, GROUP, CPS]),
                    mybir.AluOpType.mult,
                )

                gt_ps = [
                    psg.tile([F, GPB * NB * GROUP], F32, tag="g", name=f"gt{b}")
                    for b in range(BPS)
                ]

                # ---- chunk matmuls. chunk col layout: c = bucket*CPG + k,
                # bucket = b*GPB + q. start=True arms a pending-zero for the
                # whole 2KB bank on trn2: exactly once per block bank.
                for cis in range(BPS * GPB):
                    b, q = cis // GPB, cis % GPB
                    for k in range(CPG):
                        col = cis * CPG + k
                        nc.tensor.matmul(
                            gt_ps[b][:, q * (NB * GROUP) : (q + 1) * (NB * GROUP)],
                            lhsT=x_t[:, col, :],
                            rhs=msk_t[:, :, :, col],
                            start=(q == 0 and k == 0),
                            stop=(q == GPB - 1 and k == CPG - 1),
                            skip_group_check=True,
                        )

                # ---- PSUM drain: copies now, basis matmuls one SB later so
                # the PE rolls straight into the next SB's mask matmuls.
                gt_sbs = []
                for b in range(BPS):
                    gt_sb = gtp.tile([F, GPB * NB * GROUP], BF16, tag="gts", name=f"gt_sb{sb}_{b}")
                    nc.scalar.copy(gt_sb[:], gt_ps[b][:])
                    gt_sbs.append(gt_sb)
                if pending is not None:
                    basis_phase(pending[0], pending[1])
                pending = (sb, gt_sbs)
            basis_phase(pending[0], pending[1])

    nc.compile()
    return nc


def _balance(tot):
    """Pack NS nodes (total degrees tot [NS]) into NBLK*GPB groups of <=32
    nodes with per-group load <= CAP. Greedy LPT."""
    G = NBLK * GPB
    order = np.argsort(-tot, kind="stable")
    loads = np.zeros(G, np.int64)
    counts = np.zeros(G, np.int32)
    assign = np.empty(tot.shape[0], np.int32)
    slot = np.empty(tot.shape[0], np.int32)
    for n in order:
        masked = np.where(counts < GROUP, loads, 1 << 40)
        g = int(np.argmin(masked))
        assign[n] = g
        slot[n] = counts[g]
        loads[g] += tot[n]
        counts[g] += 1
    assert loads.max() <= CAP, f"group overflow: {loads.max()} > {CAP}"
    return assign, slot


def _preprocess(inp, basis_coeff, edge_val, edge_src, edge_dst):
    """Pack edges into the static (SB, chunk, slot) structure and pre-arrange
    the bf16 edge features. Returns per-core
    (xexp [NSB, 128, CPS*F] bf16, meta [128, NSB*META_COLS] bf16,
     pos2node [NBLK*BLOCK] int64)."""
    src = np.ascontiguousarray(edge_src).ravel().astype(np.int64)
    dst = np.ascontiguousarray(edge_dst).ravel().astype(np.int64)
    val = np.ascontiguousarray(edge_val).ravel().astype(np.float32)
    rel = np.repeat(np.arange(NREL, dtype=np.int64), edge_src.shape[1])
    coeff = np.asarray(basis_coeff, dtype=np.float32)  # [NREL, NB]
    inp_b = inp.astype(BF16NP)

    core = dst // NS
    per_core = []
    for c in range(NCORES):
        msel = core == c
        s_ = src[msel]
        dl = (dst[msel] - c * NS).astype(np.int64)
        v = val[msel]
        r = rel[msel]

        tot = np.bincount(dl, minlength=NS)
        assign, slot = _balance(tot)

        g = assign[dl]                           # group 0..415
        n = slot[dl].astype(np.float32)          # node slot in group, 0..31

        order = np.argsort(g, kind="stable")
        s_, v, r, n, g = (a[order] for a in (s_, v, r, n, g))
        ngr = NBLK * GPB
        cnt = np.bincount(g, minlength=ngr)
        starts = np.zeros(ngr + 1, dtype=np.int64)
        np.cumsum(cnt, out=starts[1:])
        pos = np.arange(len(s_)) - starts[g]     # 0..CAP-1 within group
        k = pos // 128                           # sub-chunk within group
        p = pos % 128                            # slot within chunk

        j = g // GPB                             # block
        q = g % GPB                              # group within block
        sbi = j // BPS                           # superblock
        col = ((j % BPS) * GPB + q) * CPG + k    # chunk col in SB, 0..63

        # ---- pre-arranged edge features (zero rows for padding slots)
        xexp_c = np.zeros((NSB, 128, CPS, F), dtype=BF16NP)
        xexp_c[sbi, p, col] = inp_b[s_]
        xexp_c = xexp_c.reshape(NSB, 128, CPS * F)

        # ---- meta [NSB, 128, META_COLS] bf16: [ldst: CPS][w4T: NB*CPS]
        mldst = np.zeros((NSB, 128, CPS), dtype=np.float32)
        mw4 = np.zeros((NSB, 128, NB, CPS), dtype=np.float32)
        mldst[sbi, p, col] = n
        mw4[sbi, p, :, col] = coeff[r] * v[:, None]
        meta_c = np.concatenate(
            [mldst, mw4.reshape(NSB, 128, NB * CPS)], axis=2
        ).astype(BF16NP)
        meta_c = np.ascontiguousarray(
            meta_c.transpose(1, 0, 2).reshape(128, NSB * META_COLS)
        )

        # ---- output permutation: (block j, q*32+n) -> node id
        pos2node = np.full(NBLK * BLOCK, -1, np.int64)
        nodes = np.arange(NS, dtype=np.int64)
        jn = assign[nodes] // GPB
        qn = assign[nodes] % GPB
        pos2node[jn * BLOCK + qn * GROUP + slot[nodes]] = nodes
        per_core.append((xexp_c, meta_c, pos2node))
    return per_core


def kernel(inp, basis_weights, basis_coeff, bias, edge_val, edge_src, edge_dst):
    inp = np.ascontiguousarray(np.asarray(inp, dtype=np.float32))
    basis_weights = np.ascontiguousarray(np.asarray(basis_weights, dtype=np.float32))
    basis_coeff = np.asarray(basis_coeff, dtype=np.float32)
    bias = np.ascontiguousarray(np.asarray(bias, dtype=np.float32))

    if "nc" not in _compiled:
        _compiled["nc"] = _build_program()
    nc = _compiled["nc"]

    per_core = _preprocess(inp, basis_coeff, edge_val, edge_src, edge_dst)
    iota_np = np.ascontiguousarray(
        np.broadcast_to(
            np.arange(GROUP, dtype=np.float32)[None, :, None], (128, GROUP, CPS)
        ).reshape(128, GROUP * CPS).astype(BF16NP)
    )
    basis_b = np.ascontiguousarray(basis_weights.astype(BF16NP))

    in_maps = []
    for c in range(NCORES):
        xexp_c, meta_c, _ = per_core[c]
        in_maps.append(
            {
                "xexp": xexp_c,
                "basisw": basis_b,
                "biasw": np.ascontiguousarray(bias.sum(0)[:, None]),
                "iota": iota_np,
                "meta": meta_c,
            }
        )

    res = run_bass_kernel_spmd(nc, in_maps, list(range(NCORES)))
    _compiled["last_results"] = res

    out = np.empty((NN, F), dtype=np.float32)
    for c in range(NCORES):
        oT = np.asarray(res.results[c]["outT"])  # [NBLK, F, BLOCK]
        rows = oT.transpose(0, 2, 1).reshape(NBLK * BLOCK, F)
        pos2node = per_core[c][2]
        valid = pos2node >= 0
        out[c * NS + pos2node[valid]] = rows[valid]
    return out
